# revision 3
# baseline (speedup 1.0000x reference)
"""Trainium2 Bass kernel for CoordsSelect (batched voxel-feature gather).

reference semantics:
  volume: [B=4, F=16, D=120, D, D] f32, coords: [B, 3*A=6144] f32,
  num_atoms: [B] int32
  vox = floor(coords_xyz) (clipped to [0,119]); flat = ix*D*D + iy*D + iz
  out[b, f, a] = volume[b, f].flat[flat[b, a]] * (a < num_atoms[b])

Strategy: host transposes each batch's volume to voxel-major [D^3, F]
so one atom's 16 features are a contiguous 64B row. Each core handles
one (batch, atom-half): 1024 atoms, mapped column-major to slots
(p, j) = atom j*128 + p. On device: compute flat voxel ids from coords
(exact floor on a wide [128, 3K] tile), add a host-provided
out-of-bounds penalty for atoms >= num_atoms, then K indirect DMAs
(nc.gpsimd.indirect_dma_start, one index per partition per call -- the
only HW-validated form) gather row flat[a] of the [D^3, 16] table into
SBUF [128, K, 16]. Invalid atoms index out of bounds and are skipped
(bounds_check, oob_is_err=False); the output tile is pre-zeroed so
they produce exact 0, matching the reference. One contiguous DMA
writes [128, K*16] back to DRAM; the host transposes while unsharding.

This replaces the baseline's per-(feature, atom) dma_gather of 256B
windows (16384 descriptors/core at ~8ns each on one Q7 pair = 131us)
with 1024 64B-row descriptors/core.
"""

import numpy as np

import concourse.bass as bass
import concourse.mybir as mybir
import concourse.tile as tile
from concourse import bacc
from concourse.bass_utils import run_bass_kernel_spmd

B, F, D = 4, 16, 120
A = 2048
D3 = D * D * D          # 1_728_000
N_CORES = 8
HALF = A // 2           # 1024 atoms per core
K = HALF // 128         # 8 gather calls (one index per partition each)

f32 = mybir.dt.float32
i32 = mybir.dt.int32
Alu = mybir.AluOpType

OOB = 4_000_000.0       # pushes invalid atoms past bounds_check


def build_bass(debug_dumps=False):
    """Build + compile the per-core Bass program (identical on all cores)."""
    nc = bacc.Bacc(
        "TRN2",
        target_bir_lowering=False,
        debug=False,
        num_devices=N_CORES,
    )

    vol = nc.dram_tensor("vol", [D3, F], f32, kind="ExternalInput")
    crd = nc.dram_tensor("crd", [3 * HALF], f32, kind="ExternalInput")
    inv = nc.dram_tensor("inv", [128, K], f32, kind="ExternalInput")
    out = nc.dram_tensor("out", [128, K * F], f32, kind="ExternalOutput")

    with tile.TileContext(nc) as tc:
        with tc.tile_pool(name="p", bufs=1) as pool:
            # gather destination: pre-zero so bounds-skipped (invalid) atoms
            # read back exact 0
            g = pool.tile([128, K, F], f32)
            nc.vector.memset(g[:], 0.0)

            # coords: partition p holds atoms {j*128 + p : j < K}, so call j
            # covers the contiguous atom range [j*128, (j+1)*128)
            crd_t = pool.tile([128, K, 3], f32)
            nc.sync.dma_start(
                crd_t[:], bass.AP(crd, 0, [[3, 128], [384, K], [1, 3]])
            )
            inv_t = pool.tile([128, K], f32)
            nc.scalar.dma_start(inv_t[:], inv.ap())

            # fl = floor(crd_t) elementwise (exact for >= 0, any cast
            # rounding mode): i = int(c); c2 = float(i); fl = c2 - (c2 > c)
            ti = pool.tile([128, K, 3], i32)
            cc = pool.tile([128, K, 3], f32)
            gt = pool.tile([128, K, 3], f32)
            fl = pool.tile([128, K, 3], f32)
            nc.vector.tensor_copy(out=ti[:], in_=crd_t[:])
            nc.vector.tensor_copy(out=cc[:], in_=ti[:])
            nc.vector.tensor_tensor(out=gt[:], in0=cc[:], in1=crd_t[:], op=Alu.is_gt)
            nc.vector.tensor_tensor(out=fl[:], in0=cc[:], in1=gt[:], op=Alu.subtract)

            # flat = fx*14400 + fy*120 + fz + inv   (exact in f32: < 2^24)
            t1 = pool.tile([128, K], f32)
            t2 = pool.tile([128, K], f32)
            acc = pool.tile([128, K], f32)
            nc.vector.tensor_scalar(
                t1[:], fl[:, :, 0:1], float(D * D), None, op0=Alu.mult
            )
            nc.vector.tensor_scalar(
                t2[:], fl[:, :, 1:2], float(D), None, op0=Alu.mult
            )
            nc.vector.tensor_tensor(out=acc[:], in0=t1[:], in1=t2[:], op=Alu.add)
            nc.vector.tensor_tensor(
                out=acc[:], in0=acc[:], in1=fl[:, :, 2:3], op=Alu.add
            )
            nc.vector.tensor_tensor(out=acc[:], in0=acc[:], in1=inv_t[:], op=Alu.add)

            idx = pool.tile([128, K], i32)
            nc.vector.tensor_copy(out=idx[:], in_=acc[:])

            # gather: g[p, j, :] = vol[idx[p, j], :] (64B row per atom)
            for j in range(K):
                nc.gpsimd.indirect_dma_start(
                    out=g[:, j, :],
                    out_offset=None,
                    in_=vol.ap(),
                    in_offset=bass.IndirectOffsetOnAxis(
                        ap=idx[:, j : j + 1], axis=0
                    ),
                    bounds_check=D3 - 1,
                    oob_is_err=False,
                )

            nc.sync.dma_start(out.ap(), g[:].rearrange("p a d -> p (a d)"))

            if debug_dumps:
                d_idx = nc.dram_tensor("d_idx", [128, K], i32, kind="ExternalOutput")
                nc.sync.dma_start(d_idx.ap(), idx[:])

    nc.compile()
    return nc


_NC_CACHE = None


def _get_nc():
    global _NC_CACHE
    if _NC_CACHE is None:
        _NC_CACHE = build_bass()
    return _NC_CACHE


def make_in_maps(volume, coords, num_atoms):
    # per-batch voxel-major volume [D^3, F] (features contiguous per voxel)
    vol_t = [
        np.ascontiguousarray(volume[b].reshape(F, D3).T) for b in range(B)
    ]
    # slot (p, j) <-> local atom j*128 + p
    local = (np.arange(K)[None, :] * 128 + np.arange(128)[:, None])  # [128, K]
    in_maps = []
    for c in range(N_CORES):
        b, h = c // 2, c % 2
        ga = h * HALF + local
        inv = np.where(ga < num_atoms[b], 0.0, OOB).astype(np.float32)
        in_maps.append(
            {
                "vol": vol_t[b],
                "crd": np.ascontiguousarray(
                    coords[b, 3 * HALF * h : 3 * HALF * (h + 1)]
                ),
                "inv": inv,
            }
        )
    return in_maps


def unshard_core(res):
    """res: [128, K*F] payload -> [F, HALF] (atom a = j*128 + p)."""
    return res.reshape(128, K, F).transpose(1, 0, 2).reshape(HALF, F).T


def kernel(volume, coords, num_atoms):
    volume = np.asarray(volume, dtype=np.float32)
    coords = np.asarray(coords, dtype=np.float32)
    num_atoms = np.asarray(num_atoms, dtype=np.int32)

    nc = _get_nc()
    in_maps = make_in_maps(volume, coords, num_atoms)
    r = run_bass_kernel_spmd(nc, in_maps, core_ids=list(range(N_CORES)))

    out = np.empty((B, F, A), dtype=np.float32)
    for c, res in enumerate(r.results):
        b, h = c // 2, c % 2
        out[b, :, h * HALF : (h + 1) * HALF] = unshard_core(res["out"])
    return out


# revision 5
# speedup vs baseline: 1.0508x; 1.0508x over previous
"""Trainium2 Bass kernel for CoordsSelect (batched voxel-feature gather).

reference semantics:
  volume: [B=4, F=16, D=120, D, D] f32, coords: [B, 3*A=6144] f32,
  num_atoms: [B] int32
  vox = floor(coords_xyz) (clipped to [0,119]); flat = ix*D*D + iy*D + iz
  out[b, f, a] = volume[b, f].flat[flat[b, a]] * (a < num_atoms[b])

Strategy: host transposes each batch's volume to voxel-major [D^3, F]
so one atom's 16 features are a contiguous 64B row. Only 128-atom
chunks that contain at least one valid atom (a < num_atoms[b]) are
processed: ceil(num_atoms/128) chunks per batch, load-balanced over
the 8 cores (each core serves a single batch; K=6 chunk slots per
core). On device: compute flat voxel ids from coords (exact floor),
add a host-provided out-of-bounds penalty for invalid atoms/slots,
then K indirect DMAs (nc.gpsimd.indirect_dma_start, one index per
partition per call -- the only HW-correct form) gather row flat[a] of
the [D^3, 16] table into SBUF [128, K, 16]. Invalid atoms index out of
bounds and are skipped (bounds_check, oob_is_err=False); the tile is
pre-zeroed so they read back exact 0. One contiguous DMA writes
[128, K*16] to DRAM; the host scatters chunks back while unsharding.

This replaces the baseline's per-(feature, atom) dma_gather of 256B
windows (16384 descriptors/core at ~8ns each on one Q7 pair = 131us)
with <=768 64B-row descriptors/core.
"""

import numpy as np

import concourse.bass as bass
import concourse.mybir as mybir
import concourse.tile as tile
from concourse import bacc
from concourse.bass_utils import run_bass_kernel_spmd

B, F, D = 4, 16, 120
A = 2048
D3 = D * D * D          # 1_728_000
N_CORES = 8

f32 = mybir.dt.float32
i32 = mybir.dt.int32
Alu = mybir.AluOpType

OOB = 4_000_000.0       # pushes invalid atoms past bounds_check


def build_bass(K, debug_dumps=False):
    """Build + compile the per-core Bass program (identical on all cores)."""
    nc = bacc.Bacc(
        "TRN2",
        target_bir_lowering=False,
        debug=False,
        num_devices=N_CORES,
    )

    vol = nc.dram_tensor("vol", [D3, F], f32, kind="ExternalInput")
    crd = nc.dram_tensor("crd", [K * 384], f32, kind="ExternalInput")
    inv = nc.dram_tensor("inv", [128, K], f32, kind="ExternalInput")
    out = nc.dram_tensor("out", [128, K * F], f32, kind="ExternalOutput")

    with tile.TileContext(nc) as tc:
        with tc.tile_pool(name="p", bufs=1) as pool:
            # coords: slot j's chunk is 384 consecutive floats; partition p
            # takes atom p of each chunk
            crd_t = pool.tile([128, K, 3], f32)
            nc.sync.dma_start(
                crd_t[:], bass.AP(crd, 0, [[3, 128], [384, K], [1, 3]])
            )
            inv_t = pool.tile([128, K], f32)
            nc.scalar.dma_start(inv_t[:], inv.ap())

            # gather destination: pre-zero so bounds-skipped (invalid) atoms
            # read back exact 0
            g = pool.tile([128, K, F], f32)
            nc.vector.memset(g[:], 0.0)

            # fl = floor(crd_t) elementwise (exact for >= 0, any cast
            # rounding mode): i = int(c); c2 = float(i); fl = c2 - (c2 > c)
            ti = pool.tile([128, K, 3], i32)
            cc = pool.tile([128, K, 3], f32)
            gt = pool.tile([128, K, 3], f32)
            fl = pool.tile([128, K, 3], f32)
            nc.vector.tensor_copy(out=ti[:], in_=crd_t[:])
            nc.vector.tensor_copy(out=cc[:], in_=ti[:])
            nc.vector.tensor_tensor(out=gt[:], in0=cc[:], in1=crd_t[:], op=Alu.is_gt)
            nc.vector.tensor_tensor(out=fl[:], in0=cc[:], in1=gt[:], op=Alu.subtract)

            # flat = fx*14400 + fy*120 + fz + inv   (exact in f32: < 2^24)
            t1 = pool.tile([128, K], f32)
            t2 = pool.tile([128, K], f32)
            acc = pool.tile([128, K], f32)
            nc.vector.tensor_scalar(
                t1[:], fl[:, :, 0:1], float(D * D), None, op0=Alu.mult
            )
            nc.vector.tensor_scalar(
                t2[:], fl[:, :, 1:2], float(D), None, op0=Alu.mult
            )
            nc.vector.tensor_tensor(out=acc[:], in0=t1[:], in1=t2[:], op=Alu.add)
            nc.vector.tensor_tensor(
                out=acc[:], in0=acc[:], in1=fl[:, :, 2:3], op=Alu.add
            )
            nc.vector.tensor_tensor(out=acc[:], in0=acc[:], in1=inv_t[:], op=Alu.add)

            idx = pool.tile([128, K], i32)
            nc.vector.tensor_copy(out=idx[:], in_=acc[:])

            # gather: g[p, j, :] = vol[idx[p, j], :] (64B row per atom)
            for j in range(K):
                nc.gpsimd.indirect_dma_start(
                    out=g[:, j, :],
                    out_offset=None,
                    in_=vol.ap(),
                    in_offset=bass.IndirectOffsetOnAxis(
                        ap=idx[:, j : j + 1], axis=0
                    ),
                    bounds_check=D3 - 1,
                    oob_is_err=False,
                )

            nc.sync.dma_start(out.ap(), g[:].rearrange("p a d -> p (a d)"))

            if debug_dumps:
                d_idx = nc.dram_tensor("d_idx", [128, K], i32, kind="ExternalOutput")
                nc.sync.dma_start(d_idx.ap(), idx[:])

    nc.compile()
    return nc


_NC_CACHE = {}


def _get_nc(K):
    if K not in _NC_CACHE:
        _NC_CACHE[K] = build_bass(K)
    return _NC_CACHE[K]


def plan_chunks(num_atoms):
    """Assign 128-atom chunks to cores. Each core serves one batch.

    Returns (K, core_batch[c], core_chunks[c] = list of chunk-lo offsets,
    padded with None up to K slots). K = max chunks on any core.
    """
    nchunks = [int(np.ceil(max(int(n), 0) / 128)) for n in num_atoms]
    # cores per batch, proportional-ish; must sum to N_CORES and give
    # every batch with work at least one core
    cores_of = [max(1, round(N_CORES * c / max(sum(nchunks), 1))) for c in nchunks]
    while sum(cores_of) > N_CORES:
        i = int(np.argmax(cores_of))
        cores_of[i] -= 1
    while sum(cores_of) < N_CORES:
        # give extra cores to the batch with the highest chunks/core
        load = [nchunks[b] / cores_of[b] for b in range(B)]
        cores_of[int(np.argmax(load))] += 1
    core_batch, parts = [], []
    for b in range(B):
        lows = [lo for lo in range(0, nchunks[b] * 128, 128)]
        nc_b = cores_of[b]
        for i in range(nc_b):
            core_batch.append(b)
            parts.append(lows[i::nc_b])
    K = max(1, max(len(p) for p in parts))
    core_chunks = [p + [None] * (K - len(p)) for p in parts]
    return K, core_batch, core_chunks


def make_in_maps(volume, coords, num_atoms):
    # per-batch voxel-major volume [D^3, F] (features contiguous per voxel)
    vol_t = [
        np.ascontiguousarray(volume[b].reshape(F, D3).T) for b in range(B)
    ]
    K, core_batch, core_chunks = plan_chunks(num_atoms)
    p = np.arange(128)
    in_maps = []
    for c in range(N_CORES):
        b = core_batch[c]
        crd = np.zeros((K, 384), dtype=np.float32)
        inv = np.full((128, K), OOB, dtype=np.float32)
        for j, lo in enumerate(core_chunks[c]):
            if lo is None:
                continue
            crd[j] = coords[b, 3 * lo : 3 * (lo + 128)]
            inv[:, j] = np.where(lo + p < num_atoms[b], 0.0, OOB)
        in_maps.append(
            {"vol": vol_t[b], "crd": crd.reshape(-1), "inv": inv}
        )
    return in_maps


def unshard(num_atoms, results):
    """Scatter per-core [128, K*F] payloads into the full [B, F, A] output."""
    K, core_batch, core_chunks = plan_chunks(num_atoms)
    out = np.zeros((B, F, A), dtype=np.float32)
    for c, res in enumerate(results):
        b = core_batch[c]
        g = res.reshape(128, K, F)
        for j, lo in enumerate(core_chunks[c]):
            if lo is None:
                continue
            out[b, :, lo : lo + 128] = g[:, j, :].T
    return out


def kernel(volume, coords, num_atoms):
    volume = np.asarray(volume, dtype=np.float32)
    coords = np.asarray(coords, dtype=np.float32)
    num_atoms = np.asarray(num_atoms, dtype=np.int32)

    K, _, _ = plan_chunks(num_atoms)
    nc = _get_nc(K)
    in_maps = make_in_maps(volume, coords, num_atoms)
    r = run_bass_kernel_spmd(nc, in_maps, core_ids=list(range(N_CORES)))
    return unshard(num_atoms, [res["out"] for res in r.results])


# revision 6
# speedup vs baseline: 1.1142x; 1.0604x over previous
"""Trainium2 Bass kernel for CoordsSelect (batched voxel-feature gather).

reference semantics:
  volume: [B=4, F=16, D=120, D, D] f32, coords: [B, 3*A=6144] f32,
  num_atoms: [B] int32
  vox = floor(coords_xyz) (clipped to [0,119]); flat = ix*D*D + iy*D + iz
  out[b, f, a] = volume[b, f].flat[flat[b, a]] * (a < num_atoms[b])

Strategy: host transposes each batch's volume to voxel-major [D^3, F]
so one atom's 16 features are a contiguous 64B row. Only 128-atom
chunks that contain at least one valid atom (a < num_atoms[b]) are
processed: ceil(num_atoms/128) chunks per batch, load-balanced over
the 8 cores (each core serves a single batch; K=6 chunk slots per
core). On device: compute flat voxel ids from coords (exact floor),
add a host-provided out-of-bounds penalty for invalid atoms/slots,
then K indirect DMAs (nc.gpsimd.indirect_dma_start, one index per
partition per call -- the only HW-correct form) gather row flat[a] of
the [D^3, 16] table into SBUF [128, K, 16]. Invalid atoms index out of
bounds and are skipped (bounds_check, oob_is_err=False); the tile is
pre-zeroed so they read back exact 0. One contiguous DMA writes
[128, K*16] to DRAM; the host scatters chunks back while unsharding.

This replaces the baseline's per-(feature, atom) dma_gather of 256B
windows (16384 descriptors/core at ~8ns each on one Q7 pair = 131us)
with <=768 64B-row descriptors/core.
"""

import numpy as np

import concourse.bass as bass
import concourse.mybir as mybir
import concourse.tile as tile
from concourse import bacc
from concourse.bass_utils import run_bass_kernel_spmd

B, F, D = 4, 16, 120
A = 2048
D3 = D * D * D          # 1_728_000
N_CORES = 8

f32 = mybir.dt.float32
i32 = mybir.dt.int32
Alu = mybir.AluOpType

OOB = 4_000_000.0       # pushes invalid atoms past bounds_check


def build_bass(K, debug_dumps=False):
    """Build + compile the per-core Bass program (identical on all cores)."""
    nc = bacc.Bacc(
        "TRN2",
        target_bir_lowering=False,
        debug=False,
        num_devices=N_CORES,
    )

    vol = nc.dram_tensor("vol", [D3, F], f32, kind="ExternalInput")
    crd = nc.dram_tensor("crd", [128, K * 3], f32, kind="ExternalInput")
    out = nc.dram_tensor("out", [128, K * F], f32, kind="ExternalOutput")

    with tile.TileContext(nc) as tc:
        with tc.tile_pool(name="p", bufs=1) as pool:
            # coords, host-transposed: partition p holds [K, 3] = slot j's
            # atom p (contiguous 12*K bytes per partition). Invalid atoms /
            # dummy slots carry (0, 0, OOB) so flat lands out of bounds.
            crd_t = pool.tile([128, K, 3], f32)
            nc.sync.dma_start(
                crd_t[:], crd.ap().rearrange("p (a d) -> p a d", d=3)
            )

            # gather destination: pre-zero so bounds-skipped (invalid) atoms
            # read back exact 0
            g = pool.tile([128, K, F], f32)
            nc.vector.memset(g[:], 0.0)

            # fl = floor(crd_t) elementwise (exact for >= 0, any cast
            # rounding mode): i = int(c); c2 = float(i); fl = c2 - (c2 > c)
            ti = pool.tile([128, K, 3], i32)
            cc = pool.tile([128, K, 3], f32)
            gt = pool.tile([128, K, 3], f32)
            fl = pool.tile([128, K, 3], f32)
            nc.vector.tensor_copy(out=ti[:], in_=crd_t[:])
            nc.vector.tensor_copy(out=cc[:], in_=ti[:])
            nc.vector.tensor_tensor(out=gt[:], in0=cc[:], in1=crd_t[:], op=Alu.is_gt)
            nc.vector.tensor_tensor(out=fl[:], in0=cc[:], in1=gt[:], op=Alu.subtract)

            # flat = fx*14400 + fy*120 + fz + inv   (exact in f32: < 2^24)
            t1 = pool.tile([128, K], f32)
            t2 = pool.tile([128, K], f32)
            acc = pool.tile([128, K], f32)
            nc.vector.tensor_scalar(
                t1[:], fl[:, :, 0:1], float(D * D), None, op0=Alu.mult
            )
            nc.vector.tensor_scalar(
                t2[:], fl[:, :, 1:2], float(D), None, op0=Alu.mult
            )
            nc.vector.tensor_tensor(out=acc[:], in0=t1[:], in1=t2[:], op=Alu.add)
            nc.vector.tensor_tensor(
                out=acc[:], in0=acc[:], in1=fl[:, :, 2:3], op=Alu.add
            )

            idx = pool.tile([128, K], i32)
            nc.vector.tensor_copy(out=idx[:], in_=acc[:])

            # gather: g[p, j, :] = vol[idx[p, j], :] (64B row per atom)
            for j in range(K):
                nc.gpsimd.indirect_dma_start(
                    out=g[:, j, :],
                    out_offset=None,
                    in_=vol.ap(),
                    in_offset=bass.IndirectOffsetOnAxis(
                        ap=idx[:, j : j + 1], axis=0
                    ),
                    bounds_check=D3 - 1,
                    oob_is_err=False,
                )

            nc.sync.dma_start(out.ap(), g[:].rearrange("p a d -> p (a d)"))

            if debug_dumps:
                d_idx = nc.dram_tensor("d_idx", [128, K], i32, kind="ExternalOutput")
                nc.sync.dma_start(d_idx.ap(), idx[:])

    nc.compile()
    return nc


_NC_CACHE = {}


def _get_nc(K):
    if K not in _NC_CACHE:
        _NC_CACHE[K] = build_bass(K)
    return _NC_CACHE[K]


def plan_chunks(num_atoms):
    """Assign 128-atom chunks to cores. Each core serves one batch.

    Returns (K, core_batch[c], core_chunks[c] = list of chunk-lo offsets,
    padded with None up to K slots). K = max chunks on any core.
    """
    nchunks = [int(np.ceil(max(int(n), 0) / 128)) for n in num_atoms]
    # cores per batch, proportional-ish; must sum to N_CORES and give
    # every batch with work at least one core
    cores_of = [max(1, round(N_CORES * c / max(sum(nchunks), 1))) for c in nchunks]
    while sum(cores_of) > N_CORES:
        i = int(np.argmax(cores_of))
        cores_of[i] -= 1
    while sum(cores_of) < N_CORES:
        # give extra cores to the batch with the highest chunks/core
        load = [nchunks[b] / cores_of[b] for b in range(B)]
        cores_of[int(np.argmax(load))] += 1
    core_batch, parts = [], []
    for b in range(B):
        lows = [lo for lo in range(0, nchunks[b] * 128, 128)]
        nc_b = cores_of[b]
        for i in range(nc_b):
            core_batch.append(b)
            parts.append(lows[i::nc_b])
    K = max(1, max(len(p) for p in parts))
    core_chunks = [p + [None] * (K - len(p)) for p in parts]
    return K, core_batch, core_chunks


def make_in_maps(volume, coords, num_atoms):
    # per-batch voxel-major volume [D^3, F] (features contiguous per voxel)
    vol_t = [
        np.ascontiguousarray(volume[b].reshape(F, D3).T) for b in range(B)
    ]
    K, core_batch, core_chunks = plan_chunks(num_atoms)
    p = np.arange(128)
    in_maps = []
    for c in range(N_CORES):
        b = core_batch[c]
        crd = np.zeros((128, K, 3), dtype=np.float32)
        crd[:, :, 2] = OOB          # dummy slots / invalid atoms -> OOB flat
        for j, lo in enumerate(core_chunks[c]):
            if lo is None:
                continue
            valid = lo + p < num_atoms[b]
            ch = coords[b, 3 * lo : 3 * (lo + 128)].reshape(128, 3)
            crd[valid, j, :] = ch[valid]
        in_maps.append({"vol": vol_t[b], "crd": crd.reshape(128, K * 3)})
    return in_maps


def unshard(num_atoms, results):
    """Scatter per-core [128, K*F] payloads into the full [B, F, A] output."""
    K, core_batch, core_chunks = plan_chunks(num_atoms)
    out = np.zeros((B, F, A), dtype=np.float32)
    for c, res in enumerate(results):
        b = core_batch[c]
        g = res.reshape(128, K, F)
        for j, lo in enumerate(core_chunks[c]):
            if lo is None:
                continue
            out[b, :, lo : lo + 128] = g[:, j, :].T
    return out


def kernel(volume, coords, num_atoms):
    volume = np.asarray(volume, dtype=np.float32)
    coords = np.asarray(coords, dtype=np.float32)
    num_atoms = np.asarray(num_atoms, dtype=np.int32)

    K, _, _ = plan_chunks(num_atoms)
    nc = _get_nc(K)
    in_maps = make_in_maps(volume, coords, num_atoms)
    r = run_bass_kernel_spmd(nc, in_maps, core_ids=list(range(N_CORES)))
    return unshard(num_atoms, [res["out"] for res in r.results])
